# revision 1
# baseline (speedup 1.0000x reference)
"""Trainium2 Bass kernel for nn_Classifier0 (quadrant-sum classifier).

reference:
    agg[n, q]  = quadrant sums of x[n] (512x512, quadrants of 256x256)
    w          = g * v[..., 0] / ||v||            [4, 4]
    y          = agg[:, :, None] * w + b_fgl      [N, 4, 4]
    out        = y.reshape(N, 16) @ W_fc.T + b_fc [N, 10]

Algebraic refactor (exact in real arithmetic):
    out[n, c] = sum_q agg[n, q] * A[q, c] + cc[c]
      A[q, c] = sum_j w[q, j] * W_fc[c, 4q + j]         (4 x 10, host, fp64)
      cc[c]   = b_fgl.ravel() @ W_fc[c] + b_fc[c]       (10, host, fp64)

Device work (data-parallel, 32 samples per core):
  - 7 C=4 chunks (samples 0..27): one contiguous 4 MB DMA each into a
    [128, 8192] tile (partition p holds 16 consecutive image rows of
    sample p // 32; p % 32 < 16 is the image's top half).  32 KB
    descriptors run the SDMA engines at ~27.0 GB/s each, 99% of the
    27.2 GB/s SBUF-AXI port rate.  Chunk 0 is split across both HWDGE
    rings (sync + scalar) so the engines spin up ~0.5 us sooner.
  - DVE tensor_reduce sums the left 256 columns of each row, ACT
    (in-place activation Copy with accum_out) sums the right 256.
  - samples 28..30: single-sample [128, 2048] chunks; sample 31 is a
    [128, 1536] chunk plus two final 128 KB strided half-row pieces
    (right then left), both reduced on DVE so the ACT accumulator-read
    latency stays off the critical tail.
  - quadrant contraction + the tiny fc = PSUM accumulation of
    zero-masked matmuls (the mask isolates the samples interleaved in
    the partition dim); the sample-31 pieces join the same psum group
    via lhsT columns that are zero except col 3.  Everything except the
    last piece's reduce + one matmul pair + copy + y DMA is hidden
    under the x stream; the post-stream serial tail is ~3.1 us.

Per-core stream is SBUF-AXI-port bound: 16 SDMA engines x ~27 GB/s
= ~432 GB/s -> 33.6 MB in ~79.4 us.  Measured window (first framework
memset -> last instruction) adds ~2.2 us body spin-up, ~3.2 us tail
and ~8.5 us of vendor-compiler epilogue (a fixed full-semaphore-file
sweep split across the five engines), landing a clean core at ~93.3 us.
On some executions roaming system HBM traffic slows the SDMA engines
10-25% (~+10-20 us); this moves between runs and cannot be countered
by layout or sharding.
"""

import numpy as np

N, S = 256, 512
H = S // 2
NCORES = 8
SPC = N // NCORES  # samples per core (32)
NCLS = 10

C = 4  # samples per DMA chunk (bulk)
NCH2 = 7  # C=4 chunks per core (samples 0..27)
NT = 3  # single-sample tail chunks (samples 28..30)
PPS = 128 // C  # partitions per sample in a C=2 chunk (64)
RPP = S // PPS  # image rows per partition (8)
FREE = S * RPP  # floats per partition per C=2 chunk (4096)
FREE1 = S * 4  # floats per partition per C=1 chunk (2048)
# sample 31 is split into a 1536-col chunk (3 rows/partition) and a final
# 512-col chunk (1 row/partition, 256 KB) so the post-stream serial chain
# (reduce -> matmul -> copy -> y DMA) is as short as possible
FA = 1536
FB = 512

_PROGRAM_CACHE = {}


def _build_program():
    from contextlib import ExitStack

    import concourse.bacc as bacc
    import concourse.mybir as mybir
    import concourse.tile as tile

    nc = bacc.Bacc("TRN2", target_bir_lowering=False, debug=False)
    dt = mybir.dt.float32

    x_t = nc.dram_tensor("x", [NCH2, 128, FREE], dt, kind="ExternalInput")
    x1_t = nc.dram_tensor("x1", [NT, 128, FREE1], dt, kind="ExternalInput")
    x31_t = nc.dram_tensor("x31", [128, FREE1], dt, kind="ExternalInput")
    # all folded params packed into one tensor: cols 0:40 walm, 40:80 warm,
    # 80:90 walm1, 90:100 warm1; row 0 cols 100:140 ccbt, 140:150 ccbt1
    cst_t = nc.dram_tensor("cst", [128, 150], dt, kind="ExternalInput")
    y_t = nc.dram_tensor("y", [SPC, NCLS], dt, kind="ExternalOutput")

    with tile.TileContext(nc) as tc, ExitStack() as ctx:
        xpool = ctx.enter_context(tc.tile_pool(name="xp", bufs=8))
        cpool = ctx.enter_context(tc.tile_pool(name="cp", bufs=1))
        ppool = ctx.enter_context(tc.tile_pool(name="pp", bufs=1, space="PSUM"))

        x_ap = x_t.ap()
        x1_ap = x1_t.ap()
        # first 28 y rows viewed as [14 chunks, 20]
        y2 = y_t.ap()[0 : C * NCH2, :].rearrange("(k j) c -> k (j c)", j=C)

        bufL = cpool.tile([128, NCH2], dt)
        bufR = cpool.tile([128, NCH2], dt)
        # tail sums: cols 0..2 = samples 28..30, col 3 = sample 31 rows 0:3
        # per partition, col 4 = sample 31 row 3 per partition
        bufL1 = cpool.tile([128, NT + 1], dt)
        bufR1 = cpool.tile([128, NT + 1], dt)
        # chunk-B lhsT: cols 0..2 stay zero so psumB2 rows 0..2 are zero and
        # the final combine is one partition-0-aligned tensor_add
        bufLB = cpool.tile([128, NT + 1], dt)
        bufRB = cpool.tile([128, NT + 1], dt)
        nc.vector.memset(bufLB[:], 0.0)
        nc.vector.memset(bufRB[:], 0.0)
        # one constant load on the scalar engine's HWDGE ring: the SP ring
        # starts streaming x immediately and GpSimd stays fully idle
        cst = cpool.tile([128, 150], dt)
        walm, warm = cst[:, 0:40], cst[:, 40:80]
        walm1, warm1 = cst[:, 80:90], cst[:, 90:100]
        ccbt, ccbt1 = cst[0:1, 100:140], cst[0:1, 140:150]
        ones1 = cpool.tile([1, NCH2], dt)
        nc.vector.memset(ones1[:], 1.0)

        for k in range(NCH2):
            xt = xpool.tile([128, FREE], dt, bufs=4)
            if k == 0:
                # split the first chunk across both HWDGE rings so the 16
                # SDMA engines spin up ~0.3 us sooner
                hf = FREE // 2
                nc.sync.dma_start(xt[:, 0:hf], x_ap[0][:, 0:hf])
                nc.scalar.dma_start(xt[:, hf:FREE], x_ap[0][:, hf:FREE])
                # constant load follows the chunk-0 half on the scalar ring
                nc.scalar.dma_start(cst[:], cst_t.ap())
            else:
                nc.sync.dma_start(xt[:], x_ap[k])
            xv = xt[:].rearrange("p (r c) -> p r c", c=S)
            nc.vector.tensor_reduce(
                bufL[:, k : k + 1],
                xv[:, :, 0:H],
                axis=mybir.AxisListType.XY,
                op=mybir.AluOpType.add,
            )
            nc.scalar.activation(
                xv[:, :, H:S],
                xv[:, :, H:S],
                mybir.ActivationFunctionType.Copy,
                accum_out=bufR[:, k : k + 1],
            )

        # single-sample tail chunks: half-size reduces on the critical tail
        for k in range(NT):
            xt1 = xpool.tile([128, FREE1], dt, tag="x1t", bufs=3)
            nc.sync.dma_start(xt1[:], x1_ap[k])
            xv1 = xt1[:].rearrange("p (r c) -> p r c", c=S)
            nc.vector.tensor_reduce(
                bufL1[:, k : k + 1],
                xv1[:, :, 0:H],
                axis=mybir.AxisListType.XY,
                op=mybir.AluOpType.add,
            )
            nc.scalar.activation(
                xv1[:, :, H:S],
                xv1[:, :, H:S],
                mybir.ActivationFunctionType.Copy,
                accum_out=bufR1[:, k : k + 1],
            )

        # sample 31, first 3 rows per partition (1.5 MB -> 0.75 MB chunk A)
        x31_ap = x31_t.ap()
        xtA = xpool.tile([128, FA], dt, tag="xAt", bufs=1)
        nc.sync.dma_start(xtA[:], x31_ap[:, 0:FA])
        xvA = xtA[:].rearrange("p (r c) -> p r c", c=S)
        nc.vector.tensor_reduce(
            bufL1[:, NT : NT + 1],
            xvA[:, :, 0:H],
            axis=mybir.AxisListType.XY,
            op=mybir.AluOpType.add,
        )
        nc.scalar.activation(
            xvA[:, :, H:S],
            xvA[:, :, H:S],
            mybir.ActivationFunctionType.Copy,
            accum_out=bufR1[:, NT : NT + 1],
        )

        # sample 31, last row per partition, split into right/left 128 KB
        # strided pieces; both reduced on DVE (no ACT accumulator-read on the
        # critical path), left piece lands last
        xtBr = xpool.tile([128, H], dt, tag="xBrt", bufs=1)
        nc.sync.dma_start(xtBr[:], x31_ap[:, FA + H : FREE1])
        xvBr = xtBr[:].rearrange("p (r c) -> p r c", c=H)
        nc.vector.tensor_reduce(
            bufRB[:, NT : NT + 1],
            xvBr[:, :, :],
            axis=mybir.AxisListType.XY,
            op=mybir.AluOpType.add,
        )
        xtBl = xpool.tile([128, H], dt, tag="xBlt", bufs=1)
        nc.sync.dma_start(xtBl[:], x31_ap[:, FA : FA + H])
        xvBl = xtBl[:].rearrange("p (r c) -> p r c", c=H)
        nc.vector.tensor_reduce(
            bufLB[:, NT : NT + 1],
            xvBl[:, :, :],
            axis=mybir.AxisListType.XY,
            op=mybir.AluOpType.add,
        )

        # C=2 chunks: all ready before the x stream drains — hidden
        psumA = ppool.tile([NCH2, C * NCLS], dt)
        nc.tensor.matmul(psumA[:], lhsT=bufL[:], rhs=walm, start=True, stop=False)
        nc.tensor.matmul(psumA[:], lhsT=bufR[:], rhs=warm, start=False, stop=False)
        nc.tensor.matmul(psumA[:], lhsT=ones1[:], rhs=ccbt, start=False, stop=True)
        outA = cpool.tile([NCH2, C * NCLS], dt)
        nc.vector.tensor_copy(outA[:], psumA[:])
        nc.sync.dma_start(y2[:], outA[:])

        # tail samples 28..31: psumB rows 0..2 = samples 28..30, row 3 =
        # sample-31 chunk A (carries the bias); psumB2 = chunk B, no bias.
        ones2 = ones1[:, 0 : NT + 1]
        psumB = ppool.tile([NT + 1, NCLS], dt)
        nc.tensor.matmul(
            psumB[:], lhsT=bufL1[:, 0 : NT + 1], rhs=walm1, start=True, stop=False
        )
        nc.tensor.matmul(
            psumB[:], lhsT=bufR1[:, 0 : NT + 1], rhs=warm1, start=False, stop=False
        )
        nc.tensor.matmul(psumB[:], lhsT=ones2, rhs=ccbt1, start=False, stop=False)
        # chunk B joins the same accumulation group; its lhsT cols 0..2 are
        # zero so only row 3 (sample 31) is affected.  RB lands before LB,
        # so the LB matmul is the only one on the critical tail.
        nc.tensor.matmul(psumB[:], lhsT=bufRB[:], rhs=warm1, start=False, stop=False)
        nc.tensor.matmul(psumB[:], lhsT=bufLB[:], rhs=walm1, start=False, stop=True)
        outB = cpool.tile([NT + 1, NCLS], dt)
        nc.vector.tensor_copy(outB[:], psumB[:])
        nc.sync.dma_start(y_t.ap()[C * NCH2 : SPC, :], outB[:])

    nc.compile()
    return nc


def _host_params(v, g, b_fgl, W_fc, b_fc):
    """Fold the tiny params into zero-masked walm/warm [128, C*10], cc [1, C*10]."""
    v64 = v.astype(np.float64)
    w = g.astype(np.float64) * (v64[..., 0] / np.linalg.norm(v64, axis=-1))  # [4,4]
    A = np.einsum("qj,cqj->qc", w, W_fc.astype(np.float64).reshape(NCLS, 4, 4))
    cc = b_fgl.astype(np.float64).reshape(-1) @ W_fc.astype(np.float64).T
    cc = cc + b_fc.astype(np.float64)

    # quadrant ids: 0=TL, 1=BL, 2=BR, 3=TR
    def masks(pps, c):
        p = np.arange(128)
        top = (p % pps) < (pps // 2)
        al_col = np.where(top[:, None], A[0][None, :], A[1][None, :])
        ar_col = np.where(top[:, None], A[3][None, :], A[2][None, :])
        grp = p // pps
        wl = np.zeros((128, c * NCLS))
        wr = np.zeros((128, c * NCLS))
        for j in range(c):
            sel = grp == j
            wl[sel, j * NCLS : (j + 1) * NCLS] = al_col[sel]
            wr[sel, j * NCLS : (j + 1) * NCLS] = ar_col[sel]
        cb = np.tile(cc, c).reshape(1, c * NCLS)
        return (
            np.ascontiguousarray(wl, dtype=np.float32),
            np.ascontiguousarray(wr, dtype=np.float32),
            np.ascontiguousarray(cb, dtype=np.float32),
        )

    return masks(PPS, C), masks(128, 1)


def _run(inputs, trace=False):
    from concourse.bass_utils import run_bass_kernel_spmd

    if "nc" not in _PROGRAM_CACHE:
        _PROGRAM_CACHE["nc"] = _build_program()
    nc = _PROGRAM_CACHE["nc"]

    x = np.ascontiguousarray(np.asarray(inputs["x"], dtype=np.float32))
    (walm, warm, ccbt), (walm1, warm1, ccbt1) = _host_params(
        np.asarray(inputs["v"], np.float32),
        np.asarray(inputs["g"], np.float32),
        np.asarray(inputs["b_fgl"], np.float32),
        np.asarray(inputs["W_fc"], np.float32),
        np.asarray(inputs["b_fc"], np.float32),
    )

    cst = np.zeros((128, 150), np.float32)
    cst[:, 0:40] = walm
    cst[:, 40:80] = warm
    cst[:, 80:90] = walm1
    cst[:, 90:100] = warm1
    cst[0, 100:140] = ccbt[0]
    cst[0, 140:150] = ccbt1[0]
    x_sh = x.reshape(NCORES, SPC * S * S)
    nb = C * NCH2 * S * S  # floats in the C=2 part
    n1 = NT * S * S  # floats in the single-sample tail part
    in_maps = [
        {
            "x": x_sh[i, :nb].reshape(NCH2, 128, FREE),
            "x1": x_sh[i, nb : nb + n1].reshape(NT, 128, FREE1),
            "x31": x_sh[i, nb + n1 :].reshape(128, FREE1),
            "cst": cst,
        }
        for i in range(NCORES)
    ]
    res = run_bass_kernel_spmd(nc, in_maps, list(range(NCORES)), trace=trace)
    y = np.concatenate([res.results[i]["y"] for i in range(NCORES)], axis=0)
    return y, res.exec_time_ns


def kernel(**inputs) -> np.ndarray:
    y, _ = _run(inputs, trace=False)
    return y

